# revision 1
# baseline (speedup 1.0000x reference)
"""DSAttention layer for Trainium2, 8 NeuronCores.

Sharding: core c -> batch b = c//2, head-group g = c%2 (4 heads each,
e-columns 256g..256g+255 of the 512-wide head dim).  tau[b]/8 (softmax
temperature x 1/sqrt(E)) is folded into each core's Wq/bq slice on the
host; delta[b] broadcasts over the softmax axis and is shift-invariant,
so it drops out exactly.  Each core emits its head-group's partial
output projection [2048, 512]; the host sums the pair per batch and
adds (bv @ Wo + bo).

Device dataflow per core (all matmul operands fp16, fp32 PSUM accum):
  X[q|k|v] [2048,512] --PE transpose--> X^T (d-major)
  qT/kT [e 256, l 2048] = W^T @ X^T   (e on partitions)
  v     [s 2048, e 256] -> fp16 v_aug [s,65] per head (ones col -> Z)
  scoresT[s,l] = kT.T @ qT  per head, head pairs concurrent via
                 partition-offset row groups (K=64 at rows 0-63/64-127)
  E = exp(scoresT - 2) fp16  (one ACT instr per [128, 2x512] pair tile)
  attnT_aug[65,l] = v_aug.T @ E  (accumulate 16 s-chunks in PSUM;
                 row 64 = softmax denominator Z)
  normalize: 1/Z (DVE) -> broadcast across partitions via K=1 matmul
                 -> attnT[64, h, l] in SBUF
  out[l,512] = sum_h attnT_h.T @ Wo_h  (K=64, accumulate 4 heads)
"""

import numpy as np
from contextlib import ExitStack

import concourse.bass as bass
import concourse.bacc as bacc
import concourse.mybir as mybir
import concourse.tile as tile
from concourse.bass_utils import run_bass_kernel_spmd
from concourse.masks import make_identity

F32 = mybir.dt.float32
F16 = mybir.dt.float16

B, L, S, D = 4, 2048, 2048, 512
H, E = 8, 64          # full model heads / head dim
HG = 4                # heads per core (head-group)
EG = HG * E           # 256, e-columns per core
N_CORES = 8

LT = L // 128         # 16 l-tiles
ST = S // 128         # 16 s-tiles
DC = D // 128         # 4 d-chunks
LQ = 4                # l-quarters of 512
SCALE = 1.0 / np.sqrt(np.float32(E))
EXP_SHIFT = -2.0      # exp(x-2): cancels in softmax, guards fp16 overflow


def _emit(ctx: ExitStack, tc: "tile.TileContext", io: dict):
    nc = tc.nc
    mm = nc.tensor.matmul

    singles = ctx.enter_context(tc.tile_pool(name="singles", bufs=1))
    bigs = ctx.enter_context(tc.tile_pool(name="bigs", bufs=1))
    xin_pool = ctx.enter_context(tc.tile_pool(name="xin", bufs=2))
    xt_pool = ctx.enter_context(tc.tile_pool(name="xt", bufs=2))
    e_pool = ctx.enter_context(tc.tile_pool(name="eslab", bufs=4))
    z_pool = ctx.enter_context(tc.tile_pool(name="zrec", bufs=2))
    ob_pool = ctx.enter_context(tc.tile_pool(name="outsb", bufs=3))

# One PSUM pool, statically 8 banks: sc 2x2 + avpj 1x2 + small 1x2.
    # "avpj" is reused: projection accumulators (prolog) then AV accumulators
    # (attention).  "small" is reused: transposes (prolog) then zb/wo psums.
    ps = ctx.enter_context(tc.tile_pool(name="ps", bufs=2, space="PSUM"))
    ps_av = ctx.enter_context(tc.tile_pool(name="ps_av", bufs=2, space="PSUM"))
    ps_wk = ctx.enter_context(tc.tile_pool(name="ps_wk", bufs=2, space="PSUM"))

    # ---- constants & weights -------------------------------------------
    ident = singles.tile([128, 128], F16)
    make_identity(nc, ident)
    ones_row = singles.tile([1, 128], F16)
    nc.vector.memset(ones_row, 1.0)
    shift_col = singles.tile([128, 1], F32)
    nc.vector.memset(shift_col, EXP_SHIFT)

    wq_sb = singles.tile([128, DC, EG], F16)   # [p, c, e] = Wq[c*128+p, e]
    wk_sb = singles.tile([128, DC, EG], F16)
    wv_sb = singles.tile([128, DC, EG], F16)
    wo_sb = singles.tile([64, HG, D], F16)     # [r, h, n] = Wo[64h+r, n]
    bq_sb = singles.tile([128, 2], F32)        # [p, ec] = bq[128ec+p]
    bk_sb = singles.tile([128, 2], F32)
    nc.sync.dma_start(out=wq_sb, in_=io["wq"][:])
    nc.sync.dma_start(out=wk_sb, in_=io["wk"][:])
    nc.sync.dma_start(out=wv_sb, in_=io["wv"][:])
    nc.sync.dma_start(out=wo_sb, in_=io["wo"][:])
    nc.sync.dma_start(out=bq_sb, in_=io["bq"][:])
    nc.sync.dma_start(out=bk_sb, in_=io["bk"][:])
    # ---- big persistent SBUF tensors -----------------------------------
    qT = bigs.tile([128, 2, L], F16, tag="qT")     # [e_in_chunk, ec, l]
    kT = bigs.tile([128, 2, S], F16, tag="kT")
    v_sb = bigs.tile([128, ST, HG, 65], F16, tag="v")  # [s_in_tile, st, h, dv+1]
    attnT = bigs.tile([64, HG, L], F16, tag="attnT")   # [e_in_head, h, l]
    nc.vector.memset(v_sb[:, :, :, 64:65], 1.0)  # ones col -> Z row

    # ---- X^T + projections ---------------------------------------------
    def load_transpose(x_dram, lc):
        """DMA X rows [512lc .. 512lc+512) and produce xt[:, c, :] =
        X^T slab [128 d, 4 l-tiles * 128] for this l-chunk."""
        xin = xin_pool.tile([128, 4, D], F32, tag="xin")
        xc = xin_pool.tile([128, 4, D], F16, tag="xc")
        for i in range(4):
            r0 = lc * 512 + i * 128
            nc.sync.dma_start(out=xin[:, i, :], in_=x_dram[r0:r0 + 128, :])
            nc.vector.tensor_copy(out=xc[:, i, :], in_=xin[:, i, :])
        xt = xt_pool.tile([128, DC, 512], F16, tag="xt")
        for c in range(DC):
            tp = ps_wk.tile([128, 512], F16, tag="work", name=f"tp_{lc}_{c}")
            for i in range(4):
                nc.tensor.transpose(tp[:, i * 128:(i + 1) * 128],
                                    xc[:, i, c * 128:(c + 1) * 128], ident)
            nc.vector.tensor_copy(out=xt[:, c, :], in_=tp)
        return xt

    def proj_qk(xt, w_sb, b_sb, dst, lc):
        # dst[:, ec, 512lc : 512lc+512] = (W.T @ X^T) + bias
        for ec in range(2):
            pp = ps_wk.tile([128, 512], F32, tag="work", name=f"pp_{lc}_{ec}")
            for c in range(DC):
                mm(pp, lhsT=w_sb[:, c, ec * 128:(ec + 1) * 128],
                   rhs=xt[:, c, :], start=(c == 0), stop=(c == DC - 1))
            nc.vector.tensor_scalar_add(
                out=dst[:, ec, lc * 512:(lc + 1) * 512], in0=pp,
                scalar1=b_sb[:, ec:ec + 1])

    def proj_v(xt, lc):
        for i in range(4):
            st = lc * 4 + i
            vp = ps_wk.tile([128, 512], F32, tag="work", name=f"vp_{lc}_{i}")[:, 0:EG]
            for c in range(DC):
                mm(vp, lhsT=xt[:, c, i * 128:(i + 1) * 128],
                   rhs=wv_sb[:, c, :], start=(c == 0), stop=(c == DC - 1))
            nc.vector.tensor_copy(
                out=v_sb[:, st, :, 0:64],
                in_=vp.rearrange("p (h e) -> p h e", h=HG))

    xt = load_transpose(io["xk"], 0)
    proj_qk(xt, wk_sb, bk_sb, kT, 0)
    xt = load_transpose(io["xq"], 0)
    proj_qk(xt, wq_sb, bq_sb, qT, 0)
    xt = load_transpose(io["xv"], 0)
    proj_v(xt, 0)
    for lc in range(1, 4):
        xt = load_transpose(io["xk"], lc)
        proj_qk(xt, wk_sb, bk_sb, kT, lc)
        xt = load_transpose(io["xv"], lc)
        proj_v(xt, lc)

    # ---- attention (q projection prefetched one l-quarter ahead) --------
    for lq in range(LQ):
        l0 = lq * 512
        if lq + 1 < LQ:
            xt = load_transpose(io["xq"], lq + 1)
            proj_qk(xt, wq_sb, bq_sb, qT, lq + 1)
        for p in range(2):                      # head pair
            av = [ps_av.tile([65, 512], F32, tag="av", name=f"av{lq}_{p}_{i}")
                  for i in range(2)]
            for j in range(ST):
                sc = ps.tile([128, 2, 512], F32, tag="sc", name=f"sc_{lq}_{p}_{j}")
                ep = e_pool.tile([128, 2, 512], F16, tag="ep")
                for hh in range(2):             # rows 0-63 / 64-127: concurrent
                    o = hh * 64
                    mm(sc[:, hh, :],
                       lhsT=kT[o:o + 64, p, j * 128:(j + 1) * 128],
                       rhs=qT[o:o + 64, p, l0:l0 + 512],
                       start=True, stop=True, tile_position=(o, 0))
                nc.scalar.activation(out=ep, in_=sc,
                                     func=mybir.ActivationFunctionType.Exp,
                                     bias=shift_col[:, 0:1], scale=1.0)
                for hh in range(2):
                    mm(av[hh], lhsT=v_sb[:, j, 2 * p + hh, :],
                       rhs=ep[:, hh, :], start=(j == 0), stop=(j == ST - 1))
            for hh in range(2):
                h = 2 * p + hh
                zrow = z_pool.tile([1, 512], F16, tag="zrow")
                nc.vector.tensor_copy(out=zrow, in_=av[hh][64:65, :])
                zcol = ps_wk.tile([128, 8], F16, tag="work", name=f"zc{lq}{p}{hh}")
                for c in range(4):
                    nc.tensor.transpose(zcol[:, 2 * c:2 * c + 1],
                                        zrow[0:1, c * 128:(c + 1) * 128],
                                        ident[0:1, 0:1])
                rcol = z_pool.tile([128, 4], F16, tag="rcol")
                with nc.allow_low_precision(reason="1/Z row scale in f16"):
                    nc.vector.reciprocal(rcol, zcol[:, 0:8:2])
                rrow = ps_wk.tile([1, 512], F16, tag="work", name=f"rr{lq}{p}{hh}")
                for c in range(4):
                    nc.tensor.transpose(rrow[0:1, c * 128:(c + 1) * 128],
                                        rcol[:, c:c + 1], ident)
                rrow_sb = z_pool.tile([1, 512], F16, tag="rrow_sb")
                nc.vector.tensor_copy(out=rrow_sb, in_=rrow)
                zb = ps_wk.tile([64, 512], F32, tag="work", name=f"zb{lq}{p}{hh}")
                mm(zb, lhsT=ones_row[0:1, 0:64], rhs=rrow_sb,
                   start=True, stop=True)
                zb_sb = z_pool.tile([64, 512], F32, tag="zb_sb")
                nc.vector.tensor_copy(out=zb_sb, in_=zb)
                nc.vector.tensor_mul(out=attnT[:, h, l0:l0 + 512],
                                     in0=av[hh][0:64, :], in1=zb_sb)
        # output projection for this l-quarter
        for i in range(4):
            lt = lq * 4 + i
            op = ps_av.tile([128, D], F32, tag="av", name=f"op_{lq}_{i}")
            for h in range(HG):
                mm(op, lhsT=attnT[:, h, lt * 128:(lt + 1) * 128],
                   rhs=wo_sb[:, h, :], start=(h == 0), stop=(h == HG - 1))
            ob = ob_pool.tile([128, D], F32, tag="ob")
            nc.vector.tensor_copy(out=ob, in_=op)
            nc.sync.dma_start(out=io["out"][lt * 128:(lt + 1) * 128, :], in_=ob)


def build_nc():
    nc = bacc.Bacc()
    io = {}
    io["xq"] = nc.declare_dram_parameter("xq", [L, D], F32, isOutput=False)
    io["xk"] = nc.declare_dram_parameter("xk", [S, D], F32, isOutput=False)
    io["xv"] = nc.declare_dram_parameter("xv", [S, D], F32, isOutput=False)
    io["wq"] = nc.declare_dram_parameter("wq", [128, DC, EG], F16, isOutput=False)
    io["wk"] = nc.declare_dram_parameter("wk", [128, DC, EG], F16, isOutput=False)
    io["wv"] = nc.declare_dram_parameter("wv", [128, DC, EG], F16, isOutput=False)
    io["wo"] = nc.declare_dram_parameter("wo", [64, HG, D], F16, isOutput=False)
    io["bq"] = nc.declare_dram_parameter("bq", [128, 2], F32, isOutput=False)
    io["bk"] = nc.declare_dram_parameter("bk", [128, 2], F32, isOutput=False)
    io["out"] = nc.declare_dram_parameter("out", [L, D], F32, isOutput=True)
    with tile.TileContext(nc) as tc:
        with ExitStack() as ctx:
            _emit(ctx, tc, io)
    nc.compile()
    return nc


_NC = None


def _get_nc():
    global _NC
    if _NC is None:
        _NC = build_nc()
    return _NC


def _chunk_w(w):
    """[512, n] -> [128, 4, n] fp16:  [p, c, :] = w[128c+p, :]"""
    n = w.shape[1]
    return np.ascontiguousarray(
        w.reshape(DC, 128, n).transpose(1, 0, 2), dtype=np.float16)


def make_in_maps(queries, keys, values, tau, Wq, bq, Wk, bk, Wv, bv, Wo):
    in_maps = []
    for c in range(N_CORES):
        b, g = c // 2, c % 2
        e0 = g * EG
        f = np.float32(SCALE * tau[b])
        wq = _chunk_w(Wq[:, e0:e0 + EG] * f)
        wk = _chunk_w(Wk[:, e0:e0 + EG])
        wv = _chunk_w(Wv[:, e0:e0 + EG])
        wo = np.ascontiguousarray(
            Wo[e0:e0 + EG, :].reshape(HG, 64, D).transpose(1, 0, 2),
            dtype=np.float16)
        in_maps.append({
            "xq": np.ascontiguousarray(queries[b], dtype=np.float32),
            "xk": np.ascontiguousarray(keys[b], dtype=np.float32),
            "xv": np.ascontiguousarray(values[b], dtype=np.float32),
            "wq": wq, "wk": wk, "wv": wv, "wo": wo,
            "bq": np.ascontiguousarray(
                (bq[e0:e0 + EG] * f).reshape(2, 128).T, dtype=np.float32),
            "bk": np.ascontiguousarray(
                bk[e0:e0 + EG].reshape(2, 128).T, dtype=np.float32),
        })
    return in_maps


def kernel(queries, keys, values, tau, delta, Wq, bq, Wk, bk, Wv, bv, Wo, bo,
           **_unused):
    queries = np.asarray(queries, dtype=np.float32)
    keys = np.asarray(keys, dtype=np.float32)
    values = np.asarray(values, dtype=np.float32)
    tau = np.asarray(tau, dtype=np.float32)
    Wq, bq = np.asarray(Wq, np.float32), np.asarray(bq, np.float32)
    Wk, bk = np.asarray(Wk, np.float32), np.asarray(bk, np.float32)
    Wv, bv = np.asarray(Wv, np.float32), np.asarray(bv, np.float32)
    Wo, bo = np.asarray(Wo, np.float32), np.asarray(bo, np.float32)

    nc = _get_nc()
    in_maps = make_in_maps(queries, keys, values, tau, Wq, bq, Wk, bk, Wv, bv, Wo)
    res = run_bass_kernel_spmd(nc, in_maps, list(range(N_CORES)))
    # attn rows sum to 1 -> +bv flows through Wo as a constant row; + bo.
    const_row = (bv @ Wo + bo).astype(np.float32)  # [512]
    out = np.empty((B, L, D), dtype=np.float32)
    for b in range(B):
        out[b] = res.results[2 * b]["out"] + res.results[2 * b + 1]["out"] \
            + const_row
    return out


if __name__ == "__main__":
    nc = build_nc()
    print("built OK")



# revision 5
# speedup vs baseline: 1.1378x; 1.1378x over previous
"""DSAttention layer for Trainium2, 8 NeuronCores.

Sharding: core c -> batch b = c//2, head-group g = c%2 (4 heads each,
e-columns 256g..256g+255 of the 512-wide head dim).  tau[b]*scale
(softmax temperature x 1/sqrt(E)) is folded into each core's Wq/bq
slice on the host; delta[b] broadcasts over the softmax axis and is
shift-invariant, so it drops out exactly.  Each core emits its
head-group's partial output projection [2048, 512] fp16; the host sums
the pair per batch and adds (bv @ Wo + bo).

Host pre-processing: X^T staged as [128, 4, 2048] fp16 (d-major), so
the device does NO transposes and all matmul operands are fp16.

Device dataflow per core:
  qT/kT [e 128, ec 2, l 2048] = W^T @ X^T   (e on partitions; bias via
      DVE tensor_scalar_add on the PSUM->SBUF move)
  v    [s 128, st 16, h 4, 65] fp16 (ones col 64 -> Z row)
  scoresT[s,l] = kT.T @ qT per head; head pairs via partition-offset
      row groups (K=64 at rows 0-63/64-127), 512-l-col blocks
  E = exp(scoresT - 2) fp16  (one ACT instr per [128, 2x512] pair tile)
  av[65, 512] += v_aug.T @ E  (16 s-chunks in PSUM; row 64 = Z)
  1/Z: DVE reciprocal on av[64:65,:] row -> rrow [1,512] f32 SBUF
  zbb[64, 512] = gpsimd partition_broadcast(rrow)  (Pool engine, SBUF)
  attnT[64hh:64hh+64, p, l] = av[0:64] * zbb  (DVE, fp16 out)
  out[l,512] = sum_p attnT[:, p, lt].T @ wo[:, p, :]  (K=128 pairs)
"""

import numpy as np
from contextlib import ExitStack

import concourse.bass as bass
import concourse.bacc as bacc
import concourse.mybir as mybir
import concourse.tile as tile
from concourse.bass_utils import run_bass_kernel_spmd

F32 = mybir.dt.float32
F16 = mybir.dt.float16

B, L, S, D = 4, 2048, 2048, 512
H, E = 8, 64          # full model heads / head dim
HG = 4                # heads per core (head-group)
EG = HG * E           # 256, e-columns per core
N_CORES = 8

ST = S // 128         # 16 s-tiles
DC = D // 128         # 4 d-chunks
LQ = 4                # l-quarters of 512
SCALE = 1.0 / np.sqrt(np.float32(E))
EXP_SHIFT = -2.0      # exp(x-2): cancels in softmax, guards fp16 overflow


def _emit(ctx: ExitStack, tc: "tile.TileContext", io: dict):
    nc = tc.nc
    mm = nc.tensor.matmul

    singles = ctx.enter_context(tc.tile_pool(name="singles", bufs=1))
    bigs = ctx.enter_context(tc.tile_pool(name="bigs", bufs=1))
    e_pool = ctx.enter_context(tc.tile_pool(name="eslab", bufs=4))
    z_pool = ctx.enter_context(tc.tile_pool(name="zrec", bufs=4))
    ob_pool = ctx.enter_context(tc.tile_pool(name="outsb", bufs=3))

    # PSUM, statically 8 banks: sc 2x2 + av 2x1 + work 2x1.
    ps_sc = ctx.enter_context(tc.tile_pool(name="ps_sc", bufs=2, space="PSUM"))
    ps_av = ctx.enter_context(tc.tile_pool(name="ps_av", bufs=2, space="PSUM"))
    ps_wk = ctx.enter_context(tc.tile_pool(name="ps_wk", bufs=2, space="PSUM"))

    # ---- constants & weights -------------------------------------------
    shift_col = singles.tile([128, 1], F32)
    nc.vector.memset(shift_col, EXP_SHIFT)

    wq_sb = singles.tile([128, DC, EG], F16)   # [p, c, e] = Wq[c*128+p, e]
    wk_sb = singles.tile([128, DC, EG], F16)
    wv_sb = singles.tile([128, DC, EG], F16)
    wo_sb = singles.tile([128, 2, D], F16)     # [r, p, n] = Wo[128p+r, n]
    bq_sb = singles.tile([128, 2], F32)        # [p, ec] = bq[128ec+p]
    bk_sb = singles.tile([128, 2], F32)
    nc.sync.dma_start(out=wq_sb, in_=io["wq"][:])
    nc.sync.dma_start(out=wk_sb, in_=io["wk"][:])
    nc.sync.dma_start(out=wv_sb, in_=io["wv"][:])
    nc.sync.dma_start(out=wo_sb, in_=io["wo"][:])
    nc.sync.dma_start(out=bq_sb, in_=io["bq"][:])
    nc.sync.dma_start(out=bk_sb, in_=io["bk"][:])

    # ---- big persistent SBUF tensors -----------------------------------
    xqT = bigs.tile([128, DC, L], F16, tag="xqT")  # [d_in_chunk, c, l]
    xkT = bigs.tile([128, DC, S], F16, tag="xkT")
    xvT = bigs.tile([128, DC, S], F16, tag="xvT")
    qT = bigs.tile([128, 2, L], F16, tag="qT")     # [e_in_chunk, ec, l]
    kT = bigs.tile([128, 2, S], F16, tag="kT")
    v_sb = bigs.tile([128, ST, HG, 65], F16, tag="v")  # [s_in_tile, st, h, e+1]
    attnT = bigs.tile([128, 2, L], F16, tag="attnT")   # [64hh+e', pair, l]
    nc.vector.memset(v_sb[:, :, :, 64:65], 1.0)  # ones col -> Z row

    # input DMAs, chunked by d so projections can start early
    for c in range(DC):
        nc.sync.dma_start(out=xqT[:, c, :], in_=io["xq"][:, c, :])
    for c in range(DC):
        nc.sync.dma_start(out=xkT[:, c, :], in_=io["xk"][:, c, :])
    for c in range(DC):
        nc.sync.dma_start(out=xvT[:, c, :], in_=io["xv"][:, c, :])

    # ---- projections -----------------------------------------------------
    def proj_qk(xt, w_sb, b_sb, dst, ec, sq):
        # dst[:, ec, 512sq : 512sq+512] = (W.T @ X^T) + bias
        pp = ps_wk.tile([128, 512], F32, tag="work", name=f"pp{ec}_{sq}")
        for c in range(DC):
            mm(pp, lhsT=w_sb[:, c, ec * 128:(ec + 1) * 128],
               rhs=xt[:, c, sq * 512:(sq + 1) * 512],
               start=(c == 0), stop=(c == DC - 1))
        nc.vector.tensor_scalar_add(
            out=dst[:, ec, sq * 512:(sq + 1) * 512], in0=pp,
            scalar1=b_sb[:, ec:ec + 1])

    def proj_v(st):
        vp = ps_wk.tile([128, 512], F32, tag="work", name=f"vp{st}")[:, 0:EG]
        for c in range(DC):
            mm(vp, lhsT=xvT[:, c, st * 128:(st + 1) * 128],
               rhs=wv_sb[:, c, :], start=(c == 0), stop=(c == DC - 1))
        nc.vector.tensor_copy(
            out=v_sb[:, st, :, 0:64],
            in_=vp.rearrange("p (h e) -> p h e", h=HG))

    # q first (ec0 covers p=0 head pair for all l), then k, then v.
    for sq in range(4):
        proj_qk(xqT, wq_sb, bq_sb, qT, 0, sq)
    for sq in range(4):
        proj_qk(xkT, wk_sb, bk_sb, kT, 0, sq)
    for sq in range(4):
        proj_qk(xqT, wq_sb, bq_sb, qT, 1, sq)
    for sq in range(4):
        proj_qk(xkT, wk_sb, bk_sb, kT, 1, sq)
    for st in range(ST):
        proj_v(st)

    # ---- attention -------------------------------------------------------
    for lq in range(LQ):
        l0 = lq * 512
        for p in range(2):                      # head pair
            av = [ps_av.tile([65, 512], F32, tag="av", name=f"av{lq}_{p}_{i}")
                  for i in range(2)]
            for j in range(ST):
                sc = ps_sc.tile([128, 2, 512], F32, tag="sc",
                                name=f"sc_{lq}_{p}_{j}")
                ep = e_pool.tile([128, 2, 512], F16, tag="ep")
                for hh in range(2):             # rows 0-63 / 64-127
                    o = hh * 64
                    mm(sc[:, hh, :],
                       lhsT=kT[o:o + 64, p, j * 128:(j + 1) * 128],
                       rhs=qT[o:o + 64, p, l0:l0 + 512],
                       start=True, stop=True, tile_position=(o, 0))
                nc.scalar.activation(out=ep, in_=sc,
                                     func=mybir.ActivationFunctionType.Exp,
                                     bias=shift_col[:, 0:1], scale=1.0)
                for hh in range(2):
                    mm(av[hh], lhsT=v_sb[:, j, 2 * p + hh, :],
                       rhs=ep[:, hh, :], start=(j == 0), stop=(j == ST - 1))
            for hh in range(2):
                rrow = z_pool.tile([1, 512], F32, tag="rrow")
                nc.vector.reciprocal(rrow, av[hh][64:65, :])
                zbb = z_pool.tile([64, 512], F32, tag="zbb")
                nc.gpsimd.partition_broadcast(zbb, rrow)
                nc.vector.tensor_mul(
                    out=attnT[64 * hh:64 * hh + 64, p, l0:l0 + 512],
                    in0=av[hh][0:64, :], in1=zbb)
        # output projection for this l-quarter (head pairs stacked, K=128)
        for i in range(4):
            lt = lq * 4 + i
            op = ps_wk.tile([128, D], F32, tag="work", name=f"op_{lq}_{i}")
            for p in range(2):
                mm(op, lhsT=attnT[:, p, lt * 128:(lt + 1) * 128],
                   rhs=wo_sb[:, p, :], start=(p == 0), stop=(p == 1))
            ob = ob_pool.tile([128, D], F16, tag="ob")
            nc.vector.tensor_copy(out=ob, in_=op)
            nc.sync.dma_start(out=io["out"][lt * 128:(lt + 1) * 128, :], in_=ob)


def build_nc():
    nc = bacc.Bacc()
    io = {}
    io["xq"] = nc.declare_dram_parameter("xq", [128, DC, L], F16, isOutput=False)
    io["xk"] = nc.declare_dram_parameter("xk", [128, DC, S], F16, isOutput=False)
    io["xv"] = nc.declare_dram_parameter("xv", [128, DC, S], F16, isOutput=False)
    io["wq"] = nc.declare_dram_parameter("wq", [128, DC, EG], F16, isOutput=False)
    io["wk"] = nc.declare_dram_parameter("wk", [128, DC, EG], F16, isOutput=False)
    io["wv"] = nc.declare_dram_parameter("wv", [128, DC, EG], F16, isOutput=False)
    io["wo"] = nc.declare_dram_parameter("wo", [128, 2, D], F16, isOutput=False)
    io["bq"] = nc.declare_dram_parameter("bq", [128, 2], F32, isOutput=False)
    io["bk"] = nc.declare_dram_parameter("bk", [128, 2], F32, isOutput=False)
    io["out"] = nc.declare_dram_parameter("out", [L, D], F16, isOutput=True)
    with tile.TileContext(nc) as tc:
        with ExitStack() as ctx:
            _emit(ctx, tc, io)
    nc.compile()
    return nc


_NC = None


def _get_nc():
    global _NC
    if _NC is None:
        _NC = build_nc()
    return _NC


def _chunk_w(w):
    """[512, n] -> [128, 4, n] fp16:  [p, c, :] = w[128c+p, :]"""
    n = w.shape[1]
    return np.ascontiguousarray(
        w.reshape(DC, 128, n).transpose(1, 0, 2), dtype=np.float16)


def _xt(x):
    """[2048, 512] f32 -> [128, 4, 2048] fp16:  [p, c, l] = x[l, 128c+p]"""
    return np.ascontiguousarray(
        x.T.reshape(DC, 128, -1).transpose(1, 0, 2), dtype=np.float16)


def make_in_maps(queries, keys, values, tau, Wq, bq, Wk, bk, Wv, bv, Wo):
    in_maps = []
    for c in range(N_CORES):
        b, g = c // 2, c % 2
        e0 = g * EG
        f = np.float32(SCALE * tau[b])
        wq = _chunk_w(Wq[:, e0:e0 + EG] * f)
        wk = _chunk_w(Wk[:, e0:e0 + EG])
        wv = _chunk_w(Wv[:, e0:e0 + EG])
        wo = np.ascontiguousarray(
            Wo[e0:e0 + EG, :].reshape(2, 128, D).transpose(1, 0, 2),
            dtype=np.float16)
        in_maps.append({
            "xq": _xt(queries[b]),
            "xk": _xt(keys[b]),
            "xv": _xt(values[b]),
            "wq": wq, "wk": wk, "wv": wv, "wo": wo,
            "bq": np.ascontiguousarray(
                (bq[e0:e0 + EG] * f).reshape(2, 128).T, dtype=np.float32),
            "bk": np.ascontiguousarray(
                bk[e0:e0 + EG].reshape(2, 128).T, dtype=np.float32),
        })
    return in_maps


def kernel(queries, keys, values, tau, delta, Wq, bq, Wk, bk, Wv, bv, Wo, bo,
           **_unused):
    queries = np.asarray(queries, dtype=np.float32)
    keys = np.asarray(keys, dtype=np.float32)
    values = np.asarray(values, dtype=np.float32)
    tau = np.asarray(tau, dtype=np.float32)
    Wq, bq = np.asarray(Wq, np.float32), np.asarray(bq, np.float32)
    Wk, bk = np.asarray(Wk, np.float32), np.asarray(bk, np.float32)
    Wv, bv = np.asarray(Wv, np.float32), np.asarray(bv, np.float32)
    Wo, bo = np.asarray(Wo, np.float32), np.asarray(bo, np.float32)

    nc = _get_nc()
    in_maps = make_in_maps(queries, keys, values, tau, Wq, bq, Wk, bk, Wv, bv, Wo)
    res = run_bass_kernel_spmd(nc, in_maps, list(range(N_CORES)))
    # attn rows sum to 1 -> +bv flows through Wo as a constant row; + bo.
    const_row = (bv @ Wo + bo).astype(np.float32)  # [512]
    out = np.empty((B, L, D), dtype=np.float32)
    for b in range(B):
        out[b] = res.results[2 * b]["out"].astype(np.float32) \
            + res.results[2 * b + 1]["out"].astype(np.float32) + const_row
    return out


if __name__ == "__main__":
    nc = build_nc()
    print("built OK")


# revision 14
# speedup vs baseline: 1.3404x; 1.1780x over previous
"""DSAttention layer for Trainium2, 8 NeuronCores.

Sharding: core c -> batch b = c//2, head-group g = c%2 (4 heads each,
e-columns 256g..256g+255 of the 512-wide head dim).  tau[b]*scale
(softmax temperature x 1/sqrt(E)) is folded into each core's Wq/bq
slice on the host; delta[b] broadcasts over the softmax axis and is
shift-invariant, so it drops out exactly.  Each core emits its
head-group's partial output projection [2048, 512] fp16; the host sums
the pair per batch and adds (bv @ Wo + bo).

Host pre-processing: X^T staged as [128, 4, 2048] fp16 (d-major), so
the device does NO transposes and all matmul operands are fp16.

Device dataflow per core:
  qT/kT [e 128, ec 2, l 2048] = W^T @ X^T   (e on partitions; bias via
      DVE tensor_scalar_add on the PSUM->SBUF move)
  v    [s 128, st 16, h 4, 65] fp16 (ones col 64 -> Z row)
  scoresT[s,l] = kT.T @ qT per head; head pairs via partition-offset
      row groups (K=64 at rows 0-63/64-127), 512-l-col blocks
  E = exp(scoresT - 2) fp16  (one ACT instr per [128, 2x512] pair tile)
  av[65, 512] += v_aug.T @ E  (16 s-chunks in PSUM; row 64 = Z)
  1/Z: DVE reciprocal on av[64:65,:] row -> rrow [1,512] f32 SBUF
  zbb[64, 512] = gpsimd partition_broadcast(rrow)  (Pool engine, SBUF)
  attnT[64hh:64hh+64, p, l] = av[0:64] * zbb  (DVE, fp16 out)
  out[l,512] = sum_p attnT[:, p, lt].T @ wo[:, p, :]  (K=128 pairs)
"""

import numpy as np
from contextlib import ExitStack

import concourse.bass as bass
import concourse.bacc as bacc
import concourse.mybir as mybir
import concourse.tile as tile
from concourse.bass_utils import run_bass_kernel_spmd

F32 = mybir.dt.float32
F16 = mybir.dt.float16

B, L, S, D = 4, 2048, 2048, 512
H, E = 8, 64          # full model heads / head dim
HG = 4                # heads per core (head-group)
EG = HG * E           # 256, e-columns per core
N_CORES = 8

ST = S // 128         # 16 s-tiles
DC = D // 128         # 4 d-chunks
LQ = 4                # l-quarters of 512
SCALE = 1.0 / np.sqrt(np.float32(E))
EXP_SHIFT = -2.0      # exp(x-2): cancels in softmax, guards fp16 overflow


def _emit(ctx: ExitStack, tc: "tile.TileContext", io: dict):
    nc = tc.nc
    mm = nc.tensor.matmul

    singles = ctx.enter_context(tc.tile_pool(name="singles", bufs=1))
    bigs = ctx.enter_context(tc.tile_pool(name="bigs", bufs=1))
    e_pool = ctx.enter_context(tc.tile_pool(name="eslab", bufs=4))
    z_pool = ctx.enter_context(tc.tile_pool(name="zrec", bufs=4))
    ob_pool = ctx.enter_context(tc.tile_pool(name="outsb", bufs=3))

    # PSUM, statically 8 banks: sc 2x2 + av 2x1 + work 2x1.
    ps_sc = ctx.enter_context(tc.tile_pool(name="ps_sc", bufs=2, space="PSUM"))
    ps_av = ctx.enter_context(tc.tile_pool(name="ps_av", bufs=2, space="PSUM"))
    ps_wk = ctx.enter_context(tc.tile_pool(name="ps_wk", bufs=2, space="PSUM"))

    # ---- constants & weights -------------------------------------------
    shift_col = singles.tile([128, 1], F32)
    nc.vector.memset(shift_col, EXP_SHIFT)

    wq_sb = singles.tile([128, DC, EG], F16)   # [p, c, e] = Wq[c*128+p, e]
    wk_sb = singles.tile([128, DC, EG], F16)
    wv_sb = singles.tile([128, DC, EG], F16)
    wo_sb = singles.tile([128, 2, D], F16)     # [r, p, n] = Wo[128p+r, n]
    bq_sb = singles.tile([128, 2], F32)        # [p, ec] = bq[128ec+p]
    bk_sb = singles.tile([128, 2], F32)

    # ---- big persistent SBUF tensors -----------------------------------
    xqT = bigs.tile([128, DC, L], F16, tag="xqT")  # [d_in_chunk, c, l]
    xkT = bigs.tile([128, DC, S], F16, tag="xkT")
    xvT = bigs.tile([128, DC, S], F16, tag="xvT")
    qT = bigs.tile([128, 2, L], F16, tag="qT")     # [e_in_chunk, ec, l]
    kT = bigs.tile([128, 2, S], F16, tag="kT")
    v_sb = bigs.tile([128, ST, HG, 65], F16, tag="v")  # [s_in_tile, st, h, e+1]
    at_pool = ctx.enter_context(tc.tile_pool(name="attnT", bufs=2))
    nc.vector.memset(v_sb[:, :, :, 64:65], 1.0)  # ones col -> Z row

    # input DMAs, chunked by d and split in l/s halves: the first halves of
    # xq/xk plus their weights gate the first score tile -- land them first.
    HL = L // 2
    for c in range(DC):
        nc.sync.dma_start(out=xqT[:, c, 0:HL], in_=io["xq"][:, c, 0:HL])
    nc.sync.dma_start(out=wq_sb, in_=io["wq"][:])
    nc.sync.dma_start(out=bq_sb, in_=io["bq"][:])
    for c in range(DC):
        nc.sync.dma_start(out=xkT[:, c, 0:HL], in_=io["xk"][:, c, 0:HL])
    nc.sync.dma_start(out=wk_sb, in_=io["wk"][:])
    nc.sync.dma_start(out=bk_sb, in_=io["bk"][:])
    nc.sync.dma_start(out=wv_sb, in_=io["wv"][:])
    for c in range(DC):
        nc.sync.dma_start(out=xvT[:, c, 0:HL], in_=io["xv"][:, c, 0:HL])
    for c in range(DC):
        nc.sync.dma_start(out=xkT[:, c, HL:S], in_=io["xk"][:, c, HL:S])
    for c in range(DC):
        nc.sync.dma_start(out=xvT[:, c, HL:S], in_=io["xv"][:, c, HL:S])
    for c in range(DC):
        nc.sync.dma_start(out=xqT[:, c, HL:L], in_=io["xq"][:, c, HL:L])
    nc.sync.dma_start(out=wo_sb, in_=io["wo"][:])

    # ---- projections -----------------------------------------------------
    def proj_qk(xt, w_sb, b_sb, dst, ec, sq):
        # dst[:, ec, 512sq : 512sq+512] = (W.T @ X^T) + bias
        pp = ps_wk.tile([128, 512], F32, tag="work", name=f"pp{ec}_{sq}")
        for c in range(DC):
            mm(pp, lhsT=w_sb[:, c, ec * 128:(ec + 1) * 128],
               rhs=xt[:, c, sq * 512:(sq + 1) * 512],
               start=(c == 0), stop=(c == DC - 1))
        nc.vector.tensor_scalar_add(
            out=dst[:, ec, sq * 512:(sq + 1) * 512], in0=pp,
            scalar1=b_sb[:, ec:ec + 1])

    def proj_v(st):
        vp = ps_wk.tile([128, 512], F32, tag="work", name=f"vp{st}")[:, 0:EG]
        for c in range(DC):
            mm(vp, lhsT=xvT[:, c, st * 128:(st + 1) * 128],
               rhs=wv_sb[:, c, :], start=(c == 0), stop=(c == DC - 1))
        nc.vector.tensor_copy(
            out=v_sb[:, st, :, 0:64],
            in_=vp.rearrange("p (h e) -> p h e", h=HG))

    # q first (ec0 covers p=0 head pair for all l), then k, then v.
    for sq in range(4):
        proj_qk(xqT, wq_sb, bq_sb, qT, 0, sq)
    for sq in range(4):
        proj_qk(xkT, wk_sb, bk_sb, kT, 0, sq)
    for sq in range(4):
        proj_qk(xqT, wq_sb, bq_sb, qT, 1, sq)
    for sq in range(4):
        proj_qk(xkT, wk_sb, bk_sb, kT, 1, sq)
    for st in range(ST):
        proj_v(st)

    # ---- attention -------------------------------------------------------
    for lq in range(LQ):
        l0 = lq * 512
        attnT = at_pool.tile([128, 2, 512], F16, tag="attnT")  # [64hh+e', pair, l]
        avs = at_pool.tile([64, HG, 512], F16, tag="avs")      # [e', h, l]
        zrs = at_pool.tile([1, HG, 512], F32, tag="zrs")       # Z rows, h in free
        for p in range(2):                      # head pair
            av = [ps_av.tile([65, 512], F32, tag="av", name=f"av{lq}_{p}_{i}")
                  for i in range(2)]
            for j in range(ST):
                sc = ps_sc.tile([128, 2, 512], F32, tag="sc",
                                name=f"sc_{lq}_{p}_{j}")
                ep = e_pool.tile([128, 2, 512], F16, tag="ep")
                for hh in range(2):             # rows 0-63 / 64-127
                    o = hh * 64
                    mm(sc[:, hh, :],
                       lhsT=kT[o:o + 64, p, j * 128:(j + 1) * 128],
                       rhs=qT[o:o + 64, p, l0:l0 + 512],
                       start=True, stop=True, tile_position=(o, 0))
                nc.scalar.activation(out=ep, in_=sc,
                                     func=mybir.ActivationFunctionType.Exp,
                                     bias=shift_col[:, 0:1], scale=1.0)
                for hh in range(2):
                    mm(av[hh], lhsT=v_sb[:, j, 2 * p + hh, :],
                       rhs=ep[:, hh, :], start=(j == 0), stop=(j == ST - 1))
            # drain av PSUM fast: numerator -> f16 SBUF, Z row -> zrs
            for hh in range(2):
                h = 2 * p + hh
                nc.vector.tensor_copy(out=zrs[:, h, :], in_=av[hh][64:65, :])
                nc.vector.tensor_copy(out=avs[:, h, :], in_=av[hh][0:64, :])
            # 1/Z off the PSUM critical path (SBUF in/out)
            for hh in range(2):
                h = 2 * p + hh
                rrow = z_pool.tile([1, 512], F32, tag="rrow")
                nc.vector.reciprocal_approx_fast(out=rrow, in_=zrs[:, h, :])
                zbb = z_pool.tile([64, 512], F32, tag="zbb")
                nc.gpsimd.partition_broadcast(zbb, rrow)
                nc.vector.tensor_mul(
                    out=attnT[64 * hh:64 * hh + 64, p, :],
                    in0=avs[:, h, :], in1=zbb)
        # output projection for this l-quarter (head pairs stacked, K=128)
        for i in range(4):
            lt = lq * 4 + i
            op = ps_wk.tile([128, D], F32, tag="work", name=f"op_{lq}_{i}")
            for p in range(2):
                mm(op, lhsT=attnT[:, p, i * 128:(i + 1) * 128],
                   rhs=wo_sb[:, p, :], start=(p == 0), stop=(p == 1))
            ob = ob_pool.tile([128, D], F16, tag="ob")
            nc.vector.tensor_copy(out=ob, in_=op)
            nc.sync.dma_start(out=io["out"][lt * 128:(lt + 1) * 128, :], in_=ob)


def build_nc():
    nc = bacc.Bacc()
    io = {}
    io["xq"] = nc.declare_dram_parameter("xq", [128, DC, L], F16, isOutput=False)
    io["xk"] = nc.declare_dram_parameter("xk", [128, DC, S], F16, isOutput=False)
    io["xv"] = nc.declare_dram_parameter("xv", [128, DC, S], F16, isOutput=False)
    io["wq"] = nc.declare_dram_parameter("wq", [128, DC, EG], F16, isOutput=False)
    io["wk"] = nc.declare_dram_parameter("wk", [128, DC, EG], F16, isOutput=False)
    io["wv"] = nc.declare_dram_parameter("wv", [128, DC, EG], F16, isOutput=False)
    io["wo"] = nc.declare_dram_parameter("wo", [128, 2, D], F16, isOutput=False)
    io["bq"] = nc.declare_dram_parameter("bq", [128, 2], F32, isOutput=False)
    io["bk"] = nc.declare_dram_parameter("bk", [128, 2], F32, isOutput=False)
    io["out"] = nc.declare_dram_parameter("out", [L, D], F16, isOutput=True)
    with tile.TileContext(nc) as tc:
        with ExitStack() as ctx:
            _emit(ctx, tc, io)
    nc.compile()
    return nc


_NC = None


def _get_nc():
    global _NC
    if _NC is None:
        _NC = build_nc()
    return _NC


def _chunk_w(w):
    """[512, n] -> [128, 4, n] fp16:  [p, c, :] = w[128c+p, :]"""
    n = w.shape[1]
    return np.ascontiguousarray(
        w.reshape(DC, 128, n).transpose(1, 0, 2), dtype=np.float16)


def _xt(x):
    """[2048, 512] f32 -> [128, 4, 2048] fp16:  [p, c, l] = x[l, 128c+p]"""
    return np.ascontiguousarray(
        x.T.reshape(DC, 128, -1).transpose(1, 0, 2), dtype=np.float16)


def make_in_maps(queries, keys, values, tau, Wq, bq, Wk, bk, Wv, bv, Wo):
    in_maps = []
    for c in range(N_CORES):
        b, g = c // 2, c % 2
        e0 = g * EG
        f = np.float32(SCALE * tau[b])
        wq = _chunk_w(Wq[:, e0:e0 + EG] * f)
        wk = _chunk_w(Wk[:, e0:e0 + EG])
        wv = _chunk_w(Wv[:, e0:e0 + EG])
        wo = np.ascontiguousarray(
            Wo[e0:e0 + EG, :].reshape(2, 128, D).transpose(1, 0, 2),
            dtype=np.float16)
        in_maps.append({
            "xq": _xt(queries[b]),
            "xk": _xt(keys[b]),
            "xv": _xt(values[b]),
            "wq": wq, "wk": wk, "wv": wv, "wo": wo,
            "bq": np.ascontiguousarray(
                (bq[e0:e0 + EG] * f).reshape(2, 128).T, dtype=np.float32),
            "bk": np.ascontiguousarray(
                bk[e0:e0 + EG].reshape(2, 128).T, dtype=np.float32),
        })
    return in_maps


def kernel(queries, keys, values, tau, delta, Wq, bq, Wk, bk, Wv, bv, Wo, bo,
           **_unused):
    queries = np.asarray(queries, dtype=np.float32)
    keys = np.asarray(keys, dtype=np.float32)
    values = np.asarray(values, dtype=np.float32)
    tau = np.asarray(tau, dtype=np.float32)
    Wq, bq = np.asarray(Wq, np.float32), np.asarray(bq, np.float32)
    Wk, bk = np.asarray(Wk, np.float32), np.asarray(bk, np.float32)
    Wv, bv = np.asarray(Wv, np.float32), np.asarray(bv, np.float32)
    Wo, bo = np.asarray(Wo, np.float32), np.asarray(bo, np.float32)

    nc = _get_nc()
    in_maps = make_in_maps(queries, keys, values, tau, Wq, bq, Wk, bk, Wv, bv, Wo)
    res = run_bass_kernel_spmd(nc, in_maps, list(range(N_CORES)))
    # attn rows sum to 1 -> +bv flows through Wo as a constant row; + bo.
    const_row = (bv @ Wo + bo).astype(np.float32)  # [512]
    out = np.empty((B, L, D), dtype=np.float32)
    for b in range(B):
        out[b] = res.results[2 * b]["out"].astype(np.float32) \
            + res.results[2 * b + 1]["out"].astype(np.float32) + const_row
    return out


if __name__ == "__main__":
    nc = build_nc()
    print("built OK")


# revision 22
# speedup vs baseline: 1.4239x; 1.0624x over previous
"""DSAttention layer for Trainium2, 8 NeuronCores.

Sharding: core c -> batch b = c//2, head-group g = c%2 (4 heads each,
e-columns 256g..256g+255 of the 512-wide head dim).  tau[b]*scale
(softmax temperature x 1/sqrt(E)) is folded into each core's Wq/bq
slice on the host; delta[b] broadcasts over the softmax axis and is
shift-invariant, so it drops out exactly.  Each core emits its
head-group's partial output projection [2048, 512] fp16; the host sums
the pair per batch and adds (bv @ Wo + bo).

Host pre-processing: X^T staged as [128, 4, 2048] fp16 (d-major), so
the device does NO transposes and all matmul operands are fp16.

Device dataflow per core:
  qT/kT [e 128, ec 2, l 2048] = W^T @ X^T   (e on partitions; bias via
      DVE tensor_scalar_add on the PSUM->SBUF move)
  v    [s 128, st 16, h 4, 65] fp16 (ones col 64 -> Z row)
  scoresT[s,l] = kT.T @ qT per head; head pairs via partition-offset
      row groups (K=64 at rows 0-63/64-127), 512-l-col blocks
  E = exp(scoresT - 2) fp16  (one ACT instr per [128, 2x512] pair tile)
  av[65, 512] += v_aug.T @ E  (16 s-chunks in PSUM; row 64 = Z)
  1/Z: DVE reciprocal on av[64:65,:] row -> rrow [1,512] f32 SBUF
  zbb[64, 512] = gpsimd partition_broadcast(rrow)  (Pool engine, SBUF)
  attnT[64hh:64hh+64, p, l] = av[0:64] * zbb  (DVE, fp16 out)
  out[l,512] = sum_p attnT[:, p, lt].T @ wo[:, p, :]  (K=128 pairs)
"""

import numpy as np
from contextlib import ExitStack

import concourse.bass as bass
import concourse.bacc as bacc
import concourse.mybir as mybir
import concourse.tile as tile
from concourse.bass_utils import run_bass_kernel_spmd

F32 = mybir.dt.float32
F16 = mybir.dt.float16

B, L, S, D = 4, 2048, 2048, 512
H, E = 8, 64          # full model heads / head dim
HG = 4                # heads per core (head-group)
EG = HG * E           # 256, e-columns per core
N_CORES = 8

ST = S // 128         # 16 s-tiles
DC = D // 128         # 4 d-chunks
LQ = 4                # l-quarters of 512
SCALE = 1.0 / np.sqrt(np.float32(E))
EXP_SHIFT = -2.0      # exp(x-2): cancels in softmax, guards fp16 overflow


def _emit(ctx: ExitStack, tc: "tile.TileContext", io: dict):
    nc = tc.nc
    mm = nc.tensor.matmul

    singles = ctx.enter_context(tc.tile_pool(name="singles", bufs=1))
    bigs = ctx.enter_context(tc.tile_pool(name="bigs", bufs=1))
    e_pool = ctx.enter_context(tc.tile_pool(name="eslab", bufs=4))
    z_pool = ctx.enter_context(tc.tile_pool(name="zrec", bufs=4))
    ob_pool = ctx.enter_context(tc.tile_pool(name="outsb", bufs=3))

    # PSUM, statically 8 banks: sc 2x2 + av 2x1 + work 2x1.
    ps_sc = ctx.enter_context(tc.tile_pool(name="ps_sc", bufs=2, space="PSUM"))
    ps_av = ctx.enter_context(tc.tile_pool(name="ps_av", bufs=2, space="PSUM"))
    ps_wk = ctx.enter_context(tc.tile_pool(name="ps_wk", bufs=2, space="PSUM"))

    # ---- constants & weights -------------------------------------------
    shift_col = singles.tile([128, 1], F32)
    nc.vector.memset(shift_col, EXP_SHIFT)

    wq_sb = singles.tile([128, DC, EG], F16)   # [p, c, e] = Wq[c*128+p, e]
    wk_sb = singles.tile([128, DC, EG], F16)
    wv_sb = singles.tile([128, DC, EG], F16)
    wo_sb = singles.tile([128, 2, D], F16)     # [r, p, n] = Wo[128p+r, n]
    bq_sb = singles.tile([128, 2], F32)        # [p, ec] = bq[128ec+p]
    bk_sb = singles.tile([128, 2], F32)

    # ---- big persistent SBUF tensors -----------------------------------
    xqT = bigs.tile([128, DC, L], F16, tag="xqT")  # [d_in_chunk, c, l]
    xkT = bigs.tile([128, DC, S], F16, tag="xkT")
    xvT = bigs.tile([128, DC, S], F16, tag="xvT")
    qT = bigs.tile([128, 2, L], F16, tag="qT")     # [e_in_chunk, ec, l]
    kT = bigs.tile([128, 2, S], F16, tag="kT")
    v_sb = bigs.tile([128, ST, HG, 65], F16, tag="v")  # [s_in_tile, st, h, e+1]
    at_pool = ctx.enter_context(tc.tile_pool(name="attnT", bufs=2))
    nc.vector.memset(v_sb[:, :, :, 64:65], 1.0)  # ones col -> Z row

    # input DMAs, chunked by d and split in l/s halves: the first halves of
    # xq/xk plus their weights gate the first score tile -- land them first.
    HL = L // 2
    for c in range(DC):
        nc.sync.dma_start(out=xqT[:, c, 0:HL], in_=io["xq"][:, c, 0:HL])
    nc.sync.dma_start(out=wq_sb, in_=io["wq"][:])
    nc.sync.dma_start(out=bq_sb, in_=io["bq"][:])
    for c in range(DC):
        nc.sync.dma_start(out=xkT[:, c, 0:HL], in_=io["xk"][:, c, 0:HL])
    nc.sync.dma_start(out=wk_sb, in_=io["wk"][:])
    nc.sync.dma_start(out=bk_sb, in_=io["bk"][:])
    nc.sync.dma_start(out=wv_sb, in_=io["wv"][:])
    for c in range(DC):
        nc.sync.dma_start(out=xvT[:, c, 0:HL], in_=io["xv"][:, c, 0:HL])
    for c in range(DC):
        nc.sync.dma_start(out=xkT[:, c, HL:S], in_=io["xk"][:, c, HL:S])
    for c in range(DC):
        nc.sync.dma_start(out=xvT[:, c, HL:S], in_=io["xv"][:, c, HL:S])
    for c in range(DC):
        nc.sync.dma_start(out=xqT[:, c, HL:L], in_=io["xq"][:, c, HL:L])
    nc.sync.dma_start(out=wo_sb, in_=io["wo"][:])

    # ---- projections -----------------------------------------------------
    def proj_qk(xt, w_sb, b_sb, dst, ec, sq):
        # dst[:, ec, 512sq : 512sq+512] = (W.T @ X^T) + bias
        pp = ps_wk.tile([128, 512], F32, tag="work", name=f"pp{ec}_{sq}")
        for c in range(DC):
            mm(pp, lhsT=w_sb[:, c, ec * 128:(ec + 1) * 128],
               rhs=xt[:, c, sq * 512:(sq + 1) * 512],
               start=(c == 0), stop=(c == DC - 1))
        nc.vector.tensor_scalar_add(
            out=dst[:, ec, sq * 512:(sq + 1) * 512], in0=pp,
            scalar1=b_sb[:, ec:ec + 1])

    def proj_v(st):
        vp = ps_wk.tile([128, 512], F32, tag="work", name=f"vp{st}")[:, 0:EG]
        for c in range(DC):
            mm(vp, lhsT=xvT[:, c, st * 128:(st + 1) * 128],
               rhs=wv_sb[:, c, :], start=(c == 0), stop=(c == DC - 1))
        nc.vector.tensor_copy(
            out=v_sb[:, st, :, 0:64],
            in_=vp.rearrange("p (h e) -> p h e", h=HG))

    # Only what gates the first score tile runs up front; the rest of the
    # projection work is queued as fillers consumed one j-step at a time so
    # the in-order PE stream never stalls on late DMA chunks.
    proj_qk(xqT, wq_sb, bq_sb, qT, 0, 0)
    proj_qk(xkT, wk_sb, bk_sb, kT, 0, 0)
    proj_v(0)
    proj_v(1)

    def P(kind, *a):
        if kind == "q":
            return lambda: proj_qk(xqT, wq_sb, bq_sb, qT, *a)
        if kind == "k":
            return lambda: proj_qk(xkT, wk_sb, bk_sb, kT, *a)
        return lambda: proj_v(*a)

    # filler schedule per (lq, p) j-loop; None = no filler this j.  The
    # slot-j filler is EMITTED AFTER the j-th av matmul, so anything read
    # at step j must sit at slot <= j-1 (v st at slot st-2 for margin).
    # need-by: k(0,sq) by (lq0,p0) j=4sq; v st by (lq0,p0) j=st;
    # k(1,sq) by (lq*,p1) j=4sq; q(ec,sq) by (lq=sq, p=ec) j=0.
    fill = {
        (0, 0): [[P("k", 0, 1), P("v", 2)], P("v", 3), P("v", 4), P("v", 5),
                 [P("k", 0, 2), P("v", 6)], P("v", 7), P("v", 8),
                 [P("k", 0, 3), P("v", 9)], P("v", 10), P("v", 11),
                 P("v", 12), P("v", 13), [P("q", 1, 0), P("v", 14)],
                 P("v", 15), None, P("k", 1, 0)],
        (0, 1): [P("k", 1, 1), P("q", 0, 1), None, None,
                 P("k", 1, 2), P("q", 0, 2), None, None,
                 P("k", 1, 3), P("q", 0, 3), None, None,
                 P("q", 1, 1), None, None, None],
        (1, 0): [P("q", 1, 2), None, None, None,
                 P("q", 1, 3), None, None, None] + [None] * 8,
    }

    # ---- attention -------------------------------------------------------
    def out_proj(attnT, lt, i):
        op = ps_wk.tile([128, D], F32, tag="work", name=f"op_{lt}")
        for p in range(2):
            mm(op, lhsT=attnT[:, p, i * 128:(i + 1) * 128],
               rhs=wo_sb[:, p, :], start=(p == 0), stop=(p == 1))
        ob = ob_pool.tile([128, D], F16, tag="ob")
        nc.vector.tensor_copy(out=ob, in_=op)
        nc.sync.dma_start(out=io["out"][lt * 128:(lt + 1) * 128, :], in_=ob)

    for lq in range(LQ):
        l0 = lq * 512
        attnT = at_pool.tile([128, 2, 512], F16, tag="attnT")  # [64hh+e', pair, l]
        avs = at_pool.tile([64, HG, 512], F16, tag="avs")      # [e', h, l]
        # Z rows staged at partition 0: reciprocal_approx_fast (custom DVE
        # uop) reads the wrong row when its input AP has a partition offset.
        zrs = at_pool.tile([1, HG, 512], F32, tag="zrs")
        for p in range(2):                      # head pair
            av = [ps_av.tile([65, 512], F32, tag="av", name=f"av{lq}_{p}_{i}")
                  for i in range(2)]
            slots = fill.get((lq, p), [None] * ST)
            for j in range(ST):
                sc = ps_sc.tile([128, 2, 512], F32, tag="sc",
                                name=f"sc_{lq}_{p}_{j}")
                ep = e_pool.tile([128, 2, 512], F16, tag="ep")
                for hh in range(2):             # rows 0-63 / 64-127
                    o = hh * 64
                    mm(sc[:, hh, :],
                       lhsT=kT[o:o + 64, p, j * 128:(j + 1) * 128],
                       rhs=qT[o:o + 64, p, l0:l0 + 512],
                       start=True, stop=True, tile_position=(o, 0))
                nc.scalar.activation(out=ep, in_=sc,
                                     func=mybir.ActivationFunctionType.Exp,
                                     bias=shift_col[:, 0:1], scale=1.0)
                for hh in range(2):
                    mm(av[hh], lhsT=v_sb[:, j, 2 * p + hh, :],
                       rhs=ep[:, hh, :], start=(j == 0), stop=(j == ST - 1))
                f = slots[j]
                if f is not None:
                    for g in (f if isinstance(f, list) else [f]):
                        g()
            # drain av PSUM fast: Z row -> zrs (partition 0), numerator -> f16
            for hh in range(2):
                h = 2 * p + hh
                nc.vector.tensor_copy(out=zrs[:, h, :], in_=av[hh][64:65, :])
                nc.vector.tensor_copy(out=avs[:, h, :], in_=av[hh][0:64, :])
            # 1/Z off the PSUM critical path (SBUF in/out)
            for hh in range(2):
                h = 2 * p + hh
                rrow = z_pool.tile([1, 512], F32, tag="rrow")
                nc.vector.reciprocal_approx_fast(out=rrow, in_=zrs[:, h, :])
                zbb = z_pool.tile([64, 512], F32, tag="zbb")
                nc.gpsimd.partition_broadcast(zbb, rrow)
                nc.vector.tensor_mul(
                    out=attnT[64 * hh:64 * hh + 64, p, :],
                    in0=avs[:, h, :], in1=zbb)
        # output projection (head pairs stacked, K=128)
        for i in range(4):
            out_proj(attnT, lq * 4 + i, i)


def build_nc():
    nc = bacc.Bacc()
    io = {}
    io["xq"] = nc.declare_dram_parameter("xq", [128, DC, L], F16, isOutput=False)
    io["xk"] = nc.declare_dram_parameter("xk", [128, DC, S], F16, isOutput=False)
    io["xv"] = nc.declare_dram_parameter("xv", [128, DC, S], F16, isOutput=False)
    io["wq"] = nc.declare_dram_parameter("wq", [128, DC, EG], F16, isOutput=False)
    io["wk"] = nc.declare_dram_parameter("wk", [128, DC, EG], F16, isOutput=False)
    io["wv"] = nc.declare_dram_parameter("wv", [128, DC, EG], F16, isOutput=False)
    io["wo"] = nc.declare_dram_parameter("wo", [128, 2, D], F16, isOutput=False)
    io["bq"] = nc.declare_dram_parameter("bq", [128, 2], F32, isOutput=False)
    io["bk"] = nc.declare_dram_parameter("bk", [128, 2], F32, isOutput=False)
    io["out"] = nc.declare_dram_parameter("out", [L, D], F16, isOutput=True)
    with tile.TileContext(nc) as tc:
        with ExitStack() as ctx:
            _emit(ctx, tc, io)
    nc.compile()
    return nc


_NC = None


def _get_nc():
    global _NC
    if _NC is None:
        _NC = build_nc()
    return _NC


def _chunk_w(w):
    """[512, n] -> [128, 4, n] fp16:  [p, c, :] = w[128c+p, :]"""
    n = w.shape[1]
    return np.ascontiguousarray(
        w.reshape(DC, 128, n).transpose(1, 0, 2), dtype=np.float16)


def _xt(x):
    """[2048, 512] f32 -> [128, 4, 2048] fp16:  [p, c, l] = x[l, 128c+p]"""
    return np.ascontiguousarray(
        x.T.reshape(DC, 128, -1).transpose(1, 0, 2), dtype=np.float16)


def make_in_maps(queries, keys, values, tau, Wq, bq, Wk, bk, Wv, bv, Wo):
    in_maps = []
    for c in range(N_CORES):
        b, g = c // 2, c % 2
        e0 = g * EG
        f = np.float32(SCALE * tau[b])
        wq = _chunk_w(Wq[:, e0:e0 + EG] * f)
        wk = _chunk_w(Wk[:, e0:e0 + EG])
        wv = _chunk_w(Wv[:, e0:e0 + EG])
        wo = np.ascontiguousarray(
            Wo[e0:e0 + EG, :].reshape(2, 128, D).transpose(1, 0, 2),
            dtype=np.float16)
        in_maps.append({
            "xq": _xt(queries[b]),
            "xk": _xt(keys[b]),
            "xv": _xt(values[b]),
            "wq": wq, "wk": wk, "wv": wv, "wo": wo,
            "bq": np.ascontiguousarray(
                (bq[e0:e0 + EG] * f).reshape(2, 128).T, dtype=np.float32),
            "bk": np.ascontiguousarray(
                bk[e0:e0 + EG].reshape(2, 128).T, dtype=np.float32),
        })
    return in_maps


def kernel(queries, keys, values, tau, delta, Wq, bq, Wk, bk, Wv, bv, Wo, bo,
           **_unused):
    queries = np.asarray(queries, dtype=np.float32)
    keys = np.asarray(keys, dtype=np.float32)
    values = np.asarray(values, dtype=np.float32)
    tau = np.asarray(tau, dtype=np.float32)
    Wq, bq = np.asarray(Wq, np.float32), np.asarray(bq, np.float32)
    Wk, bk = np.asarray(Wk, np.float32), np.asarray(bk, np.float32)
    Wv, bv = np.asarray(Wv, np.float32), np.asarray(bv, np.float32)
    Wo, bo = np.asarray(Wo, np.float32), np.asarray(bo, np.float32)

    nc = _get_nc()
    in_maps = make_in_maps(queries, keys, values, tau, Wq, bq, Wk, bk, Wv, bv, Wo)
    res = run_bass_kernel_spmd(nc, in_maps, list(range(N_CORES)))
    # attn rows sum to 1 -> +bv flows through Wo as a constant row; + bo.
    const_row = (bv @ Wo + bo).astype(np.float32)  # [512]
    out = np.empty((B, L, D), dtype=np.float32)
    for b in range(B):
        out[b] = res.results[2 * b]["out"].astype(np.float32) \
            + res.results[2 * b + 1]["out"].astype(np.float32) + const_row
    return out


if __name__ == "__main__":
    nc = build_nc()
    print("built OK")


# revision 23
# speedup vs baseline: 1.4318x; 1.0055x over previous
"""DSAttention layer for Trainium2, 8 NeuronCores.

Sharding: core c -> batch b = c//2, head-group g = c%2 (4 heads each,
e-columns 256g..256g+255 of the 512-wide head dim).  tau[b]*scale
(softmax temperature x 1/sqrt(E)) is folded into each core's Wq/bq
slice on the host; delta[b] broadcasts over the softmax axis and is
shift-invariant, so it drops out exactly.  Each core emits its
head-group's partial output projection [2048, 512] fp16; the host sums
the pair per batch and adds (bv @ Wo + bo).

Host pre-processing: X^T staged as [128, 4, 2048] fp16 (d-major), so
the device does NO transposes and all matmul operands are fp16.

Device dataflow per core:
  qT/kT [e 128, ec 2, l 2048] = W^T @ X^T   (e on partitions; bias via
      DVE tensor_scalar_add on the PSUM->SBUF move)
  v    [s 128, st 16, h 4, 65] fp16 (ones col 64 -> Z row)
  scoresT[s,l] = kT.T @ qT per head; head pairs via partition-offset
      row groups (K=64 at rows 0-63/64-127), 512-l-col blocks
  E = exp(scoresT - 2) fp16  (one ACT instr per [128, 2x512] pair tile)
  av[65, 512] += v_aug.T @ E  (16 s-chunks in PSUM; row 64 = Z)
  1/Z: DVE reciprocal on av[64:65,:] row -> rrow [1,512] f32 SBUF
  zbb[64, 512] = gpsimd partition_broadcast(rrow)  (Pool engine, SBUF)
  attnT[64hh:64hh+64, p, l] = av[0:64] * zbb  (DVE, fp16 out)
  out[l,512] = sum_p attnT[:, p, lt].T @ wo[:, p, :]  (K=128 pairs)
"""

import numpy as np
from contextlib import ExitStack

import concourse.bass as bass
import concourse.bacc as bacc
import concourse.mybir as mybir
import concourse.tile as tile
from concourse.bass_utils import run_bass_kernel_spmd

F32 = mybir.dt.float32
F16 = mybir.dt.float16

B, L, S, D = 4, 2048, 2048, 512
H, E = 8, 64          # full model heads / head dim
HG = 4                # heads per core (head-group)
EG = HG * E           # 256, e-columns per core
N_CORES = 8

ST = S // 128         # 16 s-tiles
DC = D // 128         # 4 d-chunks
LQ = 4                # l-quarters of 512
SCALE = 1.0 / np.sqrt(np.float32(E))
EXP_SHIFT = -2.0      # exp(x-2): cancels in softmax, guards fp16 overflow


def _emit(ctx: ExitStack, tc: "tile.TileContext", io: dict):
    nc = tc.nc
    mm = nc.tensor.matmul

    singles = ctx.enter_context(tc.tile_pool(name="singles", bufs=1))
    bigs = ctx.enter_context(tc.tile_pool(name="bigs", bufs=1))
    e_pool = ctx.enter_context(tc.tile_pool(name="eslab", bufs=4))
    z_pool = ctx.enter_context(tc.tile_pool(name="zrec", bufs=4))
    ob_pool = ctx.enter_context(tc.tile_pool(name="outsb", bufs=3))

    # PSUM, statically 8 banks: sc 2x2 + av 2x1 + work 2x1.
    ps_sc = ctx.enter_context(tc.tile_pool(name="ps_sc", bufs=2, space="PSUM"))
    ps_av = ctx.enter_context(tc.tile_pool(name="ps_av", bufs=2, space="PSUM"))
    ps_wk = ctx.enter_context(tc.tile_pool(name="ps_wk", bufs=2, space="PSUM"))

    # ---- constants & weights -------------------------------------------
    shift_col = singles.tile([128, 1], F32)
    nc.vector.memset(shift_col, EXP_SHIFT)

    wq_sb = singles.tile([128, DC, EG], F16)   # [p, c, e] = Wq[c*128+p, e]
    wk_sb = singles.tile([128, DC, EG], F16)
    wv_sb = singles.tile([128, DC, EG], F16)
    wo_sb = singles.tile([128, 2, D], F16)     # [r, p, n] = Wo[128p+r, n]
    bq_sb = singles.tile([128, 2], F32)        # [p, ec] = bq[128ec+p]
    bk_sb = singles.tile([128, 2], F32)

    # ---- big persistent SBUF tensors -----------------------------------
    xqT = bigs.tile([128, DC, L], F16, tag="xqT")  # [d_in_chunk, c, l]
    xkT = bigs.tile([128, DC, S], F16, tag="xkT")
    xvT = bigs.tile([128, DC, S], F16, tag="xvT")
    qT = bigs.tile([128, 2, L], F16, tag="qT")     # [e_in_chunk, ec, l]
    kT = bigs.tile([128, 2, S], F16, tag="kT")
    v_sb = bigs.tile([128, ST, HG, 65], F16, tag="v")  # [s_in_tile, st, h, e+1]
    at_pool = ctx.enter_context(tc.tile_pool(name="attnT", bufs=2))
    nc.vector.memset(v_sb[:, :, :, 64:65], 1.0)  # ones col -> Z row

    # input DMAs, chunked by d and sliced along l/s, ordered by need time:
    # the first quarter of xq/xk plus weights gates the first score tile.
    Q1, H1 = L // 4, L // 2

    def dma_x(dst, src, a, b):
        for c in range(DC):
            nc.sync.dma_start(out=dst[:, c, a:b], in_=src[:, c, a:b])

    dma_x(xqT, io["xq"], 0, Q1)
    nc.sync.dma_start(out=wq_sb, in_=io["wq"][:])
    nc.sync.dma_start(out=bq_sb, in_=io["bq"][:])
    dma_x(xkT, io["xk"], 0, Q1)
    nc.sync.dma_start(out=wk_sb, in_=io["wk"][:])
    nc.sync.dma_start(out=bk_sb, in_=io["bk"][:])
    nc.sync.dma_start(out=wv_sb, in_=io["wv"][:])
    dma_x(xvT, io["xv"], 0, Q1)      # v st0-3
    dma_x(xkT, io["xk"], Q1, H1)     # scores j4-7
    dma_x(xvT, io["xv"], Q1, H1)     # v st4-7
    dma_x(xvT, io["xv"], H1, S)      # v st8-15
    dma_x(xkT, io["xk"], H1, S)      # scores j8-15
    dma_x(xqT, io["xq"], Q1, H1)     # q(0,1)/q(1,1) for lq1
    dma_x(xqT, io["xq"], H1, L)      # lq2-3
    nc.sync.dma_start(out=wo_sb, in_=io["wo"][:])

    # ---- projections -----------------------------------------------------
    def proj_qk(xt, w_sb, b_sb, dst, ec, sq):
        # dst[:, ec, 512sq : 512sq+512] = (W.T @ X^T) + bias
        pp = ps_wk.tile([128, 512], F32, tag="work", name=f"pp{ec}_{sq}")
        for c in range(DC):
            mm(pp, lhsT=w_sb[:, c, ec * 128:(ec + 1) * 128],
               rhs=xt[:, c, sq * 512:(sq + 1) * 512],
               start=(c == 0), stop=(c == DC - 1))
        nc.vector.tensor_scalar_add(
            out=dst[:, ec, sq * 512:(sq + 1) * 512], in0=pp,
            scalar1=b_sb[:, ec:ec + 1])

    def proj_v(st):
        vp = ps_wk.tile([128, 512], F32, tag="work", name=f"vp{st}")[:, 0:EG]
        for c in range(DC):
            mm(vp, lhsT=xvT[:, c, st * 128:(st + 1) * 128],
               rhs=wv_sb[:, c, :], start=(c == 0), stop=(c == DC - 1))
        nc.vector.tensor_copy(
            out=v_sb[:, st, :, 0:64],
            in_=vp.rearrange("p (h e) -> p h e", h=HG))

    # Only what gates the first score tile runs up front; the rest of the
    # projection work is queued as fillers consumed one j-step at a time so
    # the in-order PE stream never stalls on late DMA chunks.
    proj_qk(xqT, wq_sb, bq_sb, qT, 0, 0)
    proj_qk(xkT, wk_sb, bk_sb, kT, 0, 0)
    proj_v(0)
    proj_v(1)

    def P(kind, *a):
        if kind == "q":
            return lambda: proj_qk(xqT, wq_sb, bq_sb, qT, *a)
        if kind == "k":
            return lambda: proj_qk(xkT, wk_sb, bk_sb, kT, *a)
        return lambda: proj_v(*a)

    # filler schedule per (lq, p) j-loop; None = no filler this j.  The
    # slot-j filler is EMITTED AFTER the j-th av matmul, so anything read
    # at step j must sit at slot <= j-1 (v st at slot st-2 for margin).
    # need-by: k(0,sq) by (lq0,p0) j=4sq; v st by (lq0,p0) j=st;
    # k(1,sq) by (lq*,p1) j=4sq; q(ec,sq) by (lq=sq, p=ec) j=0.
    fill = {
        (0, 0): [[P("k", 0, 1), P("v", 2)], P("v", 3), P("v", 4), P("v", 5),
                 [P("k", 0, 2), P("v", 6)], P("v", 7), P("v", 8),
                 [P("k", 0, 3), P("v", 9)], P("v", 10), P("v", 11),
                 P("v", 12), P("v", 13), [P("q", 1, 0), P("v", 14)],
                 P("v", 15), None, P("k", 1, 0)],
        (0, 1): [P("k", 1, 1), P("q", 0, 1), None, None,
                 P("k", 1, 2), P("q", 0, 2), None, None,
                 P("k", 1, 3), P("q", 0, 3), None, None,
                 P("q", 1, 1), None, None, None],
        (1, 0): [P("q", 1, 2), None, None, None,
                 P("q", 1, 3), None, None, None] + [None] * 8,
    }

    # ---- attention -------------------------------------------------------
    def out_proj(attnT, lt, i):
        op = ps_wk.tile([128, D], F32, tag="work", name=f"op_{lt}")
        for p in range(2):
            mm(op, lhsT=attnT[:, p, i * 128:(i + 1) * 128],
               rhs=wo_sb[:, p, :], start=(p == 0), stop=(p == 1))
        ob = ob_pool.tile([128, D], F16, tag="ob")
        nc.vector.tensor_copy(out=ob, in_=op)
        nc.sync.dma_start(out=io["out"][lt * 128:(lt + 1) * 128, :], in_=ob)

    for lq in range(LQ):
        l0 = lq * 512
        attnT = at_pool.tile([128, 2, 512], F16, tag="attnT")  # [64hh+e', pair, l]
        avs = at_pool.tile([64, HG, 512], F16, tag="avs")      # [e', h, l]
        # Z rows staged at partition 0: reciprocal_approx_fast (custom DVE
        # uop) reads the wrong row when its input AP has a partition offset.
        zrs = at_pool.tile([1, HG, 512], F32, tag="zrs")
        for p in range(2):                      # head pair
            av = [ps_av.tile([65, 512], F32, tag="av", name=f"av{lq}_{p}_{i}")
                  for i in range(2)]
            slots = fill.get((lq, p), [None] * ST)
            for j in range(ST):
                sc = ps_sc.tile([128, 2, 512], F32, tag="sc",
                                name=f"sc_{lq}_{p}_{j}")
                ep = e_pool.tile([128, 2, 512], F16, tag="ep")
                for hh in range(2):             # rows 0-63 / 64-127
                    o = hh * 64
                    mm(sc[:, hh, :],
                       lhsT=kT[o:o + 64, p, j * 128:(j + 1) * 128],
                       rhs=qT[o:o + 64, p, l0:l0 + 512],
                       start=True, stop=True, tile_position=(o, 0))
                nc.scalar.activation(out=ep, in_=sc,
                                     func=mybir.ActivationFunctionType.Exp,
                                     bias=shift_col[:, 0:1], scale=1.0)
                for hh in range(2):
                    mm(av[hh], lhsT=v_sb[:, j, 2 * p + hh, :],
                       rhs=ep[:, hh, :], start=(j == 0), stop=(j == ST - 1))
                f = slots[j]
                if f is not None:
                    for g in (f if isinstance(f, list) else [f]):
                        g()
            # drain av PSUM fast: Z row -> zrs (partition 0), numerator -> f16
            for hh in range(2):
                h = 2 * p + hh
                nc.vector.tensor_copy(out=zrs[:, h, :], in_=av[hh][64:65, :])
                nc.vector.tensor_copy(out=avs[:, h, :], in_=av[hh][0:64, :])
            # 1/Z off the PSUM critical path (SBUF in/out)
            for hh in range(2):
                h = 2 * p + hh
                rrow = z_pool.tile([1, 512], F32, tag="rrow")
                nc.vector.reciprocal_approx_fast(out=rrow, in_=zrs[:, h, :])
                zbb = z_pool.tile([64, 512], F32, tag="zbb")
                nc.gpsimd.partition_broadcast(zbb, rrow)
                nc.vector.tensor_mul(
                    out=attnT[64 * hh:64 * hh + 64, p, :],
                    in0=avs[:, h, :], in1=zbb)
        # output projection (head pairs stacked, K=128)
        for i in range(4):
            out_proj(attnT, lq * 4 + i, i)


def build_nc():
    nc = bacc.Bacc()
    io = {}
    io["xq"] = nc.declare_dram_parameter("xq", [128, DC, L], F16, isOutput=False)
    io["xk"] = nc.declare_dram_parameter("xk", [128, DC, S], F16, isOutput=False)
    io["xv"] = nc.declare_dram_parameter("xv", [128, DC, S], F16, isOutput=False)
    io["wq"] = nc.declare_dram_parameter("wq", [128, DC, EG], F16, isOutput=False)
    io["wk"] = nc.declare_dram_parameter("wk", [128, DC, EG], F16, isOutput=False)
    io["wv"] = nc.declare_dram_parameter("wv", [128, DC, EG], F16, isOutput=False)
    io["wo"] = nc.declare_dram_parameter("wo", [128, 2, D], F16, isOutput=False)
    io["bq"] = nc.declare_dram_parameter("bq", [128, 2], F32, isOutput=False)
    io["bk"] = nc.declare_dram_parameter("bk", [128, 2], F32, isOutput=False)
    io["out"] = nc.declare_dram_parameter("out", [L, D], F16, isOutput=True)
    with tile.TileContext(nc) as tc:
        with ExitStack() as ctx:
            _emit(ctx, tc, io)
    nc.compile()
    return nc


_NC = None


def _get_nc():
    global _NC
    if _NC is None:
        _NC = build_nc()
    return _NC


def _chunk_w(w):
    """[512, n] -> [128, 4, n] fp16:  [p, c, :] = w[128c+p, :]"""
    n = w.shape[1]
    return np.ascontiguousarray(
        w.reshape(DC, 128, n).transpose(1, 0, 2), dtype=np.float16)


def _xt(x):
    """[2048, 512] f32 -> [128, 4, 2048] fp16:  [p, c, l] = x[l, 128c+p]"""
    return np.ascontiguousarray(
        x.T.reshape(DC, 128, -1).transpose(1, 0, 2), dtype=np.float16)


def make_in_maps(queries, keys, values, tau, Wq, bq, Wk, bk, Wv, bv, Wo):
    in_maps = []
    for c in range(N_CORES):
        b, g = c // 2, c % 2
        e0 = g * EG
        f = np.float32(SCALE * tau[b])
        wq = _chunk_w(Wq[:, e0:e0 + EG] * f)
        wk = _chunk_w(Wk[:, e0:e0 + EG])
        wv = _chunk_w(Wv[:, e0:e0 + EG])
        wo = np.ascontiguousarray(
            Wo[e0:e0 + EG, :].reshape(2, 128, D).transpose(1, 0, 2),
            dtype=np.float16)
        in_maps.append({
            "xq": _xt(queries[b]),
            "xk": _xt(keys[b]),
            "xv": _xt(values[b]),
            "wq": wq, "wk": wk, "wv": wv, "wo": wo,
            "bq": np.ascontiguousarray(
                (bq[e0:e0 + EG] * f).reshape(2, 128).T, dtype=np.float32),
            "bk": np.ascontiguousarray(
                bk[e0:e0 + EG].reshape(2, 128).T, dtype=np.float32),
        })
    return in_maps


def kernel(queries, keys, values, tau, delta, Wq, bq, Wk, bk, Wv, bv, Wo, bo,
           **_unused):
    queries = np.asarray(queries, dtype=np.float32)
    keys = np.asarray(keys, dtype=np.float32)
    values = np.asarray(values, dtype=np.float32)
    tau = np.asarray(tau, dtype=np.float32)
    Wq, bq = np.asarray(Wq, np.float32), np.asarray(bq, np.float32)
    Wk, bk = np.asarray(Wk, np.float32), np.asarray(bk, np.float32)
    Wv, bv = np.asarray(Wv, np.float32), np.asarray(bv, np.float32)
    Wo, bo = np.asarray(Wo, np.float32), np.asarray(bo, np.float32)

    nc = _get_nc()
    in_maps = make_in_maps(queries, keys, values, tau, Wq, bq, Wk, bk, Wv, bv, Wo)
    res = run_bass_kernel_spmd(nc, in_maps, list(range(N_CORES)))
    # attn rows sum to 1 -> +bv flows through Wo as a constant row; + bo.
    const_row = (bv @ Wo + bo).astype(np.float32)  # [512]
    out = np.empty((B, L, D), dtype=np.float32)
    for b in range(B):
        out[b] = res.results[2 * b]["out"].astype(np.float32) \
            + res.results[2 * b + 1]["out"].astype(np.float32) + const_row
    return out


if __name__ == "__main__":
    nc = build_nc()
    print("built OK")


# revision 25
# speedup vs baseline: 1.4370x; 1.0036x over previous
"""DSAttention layer for Trainium2, 8 NeuronCores.

Sharding: core c -> batch b = c//2, head-group g = c%2 (4 heads each,
e-columns 256g..256g+255 of the 512-wide head dim).  tau[b]*scale
(softmax temperature x 1/sqrt(E)) is folded into each core's Wq/bq
slice on the host; delta[b] broadcasts over the softmax axis and is
shift-invariant, so it drops out exactly.  Each core emits its
head-group's partial output projection [2048, 512] fp16; the host sums
the pair per batch and adds (bv @ Wo + bo).

Host pre-processing: X^T staged as [128, 4, 2048] fp16 (d-major), so
the device does NO transposes and all matmul operands are fp16.

Device dataflow per core:
  qT/kT [e 128, ec 2, l 2048] = W^T @ X^T   (e on partitions; bias via
      DVE tensor_scalar_add on the PSUM->SBUF move)
  v    [s 128, st 16, h 4, 65] fp16 (ones col 64 -> Z row)
  scoresT[s,l] = kT.T @ qT per head; head pairs via partition-offset
      row groups (K=64 at rows 0-63/64-127), 512-l-col blocks
  E = exp(scoresT - 2) fp16  (one ACT instr per [128, 2x512] pair tile)
  av[65, 512] += v_aug.T @ E  (16 s-chunks in PSUM; row 64 = Z)
  1/Z: DVE reciprocal on av[64:65,:] row -> rrow [1,512] f32 SBUF
  zbb[64, 512] = gpsimd partition_broadcast(rrow)  (Pool engine, SBUF)
  attnT[64hh:64hh+64, p, l] = av[0:64] * zbb  (DVE, fp16 out)
  out[l,512] = sum_p attnT[:, p, lt].T @ wo[:, p, :]  (K=128 pairs)
"""

import numpy as np
from contextlib import ExitStack

import concourse.bass as bass
import concourse.bacc as bacc
import concourse.mybir as mybir
import concourse.tile as tile
from concourse.bass_utils import run_bass_kernel_spmd

F32 = mybir.dt.float32
F16 = mybir.dt.float16

B, L, S, D = 4, 2048, 2048, 512
H, E = 8, 64          # full model heads / head dim
HG = 4                # heads per core (head-group)
EG = HG * E           # 256, e-columns per core
N_CORES = 8

ST = S // 128         # 16 s-tiles
DC = D // 128         # 4 d-chunks
LQ = 4                # l-quarters of 512
SCALE = 1.0 / np.sqrt(np.float32(E))
EXP_SHIFT = -2.0      # exp(x-2): cancels in softmax, guards fp16 overflow


def _emit(ctx: ExitStack, tc: "tile.TileContext", io: dict):
    nc = tc.nc
    mm = nc.tensor.matmul

    singles = ctx.enter_context(tc.tile_pool(name="singles", bufs=1))
    bigs = ctx.enter_context(tc.tile_pool(name="bigs", bufs=1))
    e_pool = ctx.enter_context(tc.tile_pool(name="eslab", bufs=4))
    z_pool = ctx.enter_context(tc.tile_pool(name="zrec", bufs=4))
    ob_pool = ctx.enter_context(tc.tile_pool(name="outsb", bufs=3))

    # PSUM, statically 8 banks: sc 2x2 + av 2x1 + work 2x1.
    ps_sc = ctx.enter_context(tc.tile_pool(name="ps_sc", bufs=2, space="PSUM"))
    ps_av = ctx.enter_context(tc.tile_pool(name="ps_av", bufs=2, space="PSUM"))
    ps_wk = ctx.enter_context(tc.tile_pool(name="ps_wk", bufs=2, space="PSUM"))

    # ---- constants & weights -------------------------------------------
    shift_col = singles.tile([128, 1], F32)
    nc.vector.memset(shift_col, EXP_SHIFT)

    wq_sb = singles.tile([128, DC, EG], F16)   # [p, c, e] = Wq[c*128+p, e]
    wk_sb = singles.tile([128, DC, EG], F16)
    wv_sb = singles.tile([128, DC, EG], F16)
    wo_sb = singles.tile([128, 2, D], F16)     # [r, p, n] = Wo[128p+r, n]
    bq_sb = singles.tile([128, 2], F32)        # [p, ec] = bq[128ec+p]
    bk_sb = singles.tile([128, 2], F32)

    # ---- big persistent SBUF tensors -----------------------------------
    xqT = bigs.tile([128, DC, L], F16, tag="xqT")  # [d_in_chunk, c, l]
    xkT = bigs.tile([128, DC, S], F16, tag="xkT")
    xvT = bigs.tile([128, DC, S], F16, tag="xvT")
    qT = bigs.tile([128, 2, L], F16, tag="qT")     # [e_in_chunk, ec, l]
    kT = bigs.tile([128, 2, S], F16, tag="kT")
    v_sb = bigs.tile([128, ST, HG, 65], F16, tag="v")  # [s_in_tile, st, h, e+1]
    at_pool = ctx.enter_context(tc.tile_pool(name="attnT", bufs=2))
    nc.vector.memset(v_sb[:, :, :, 64:65], 1.0)  # ones col -> Z row

    # input DMAs, chunked by d and sliced along l/s, ordered by need time:
    # the first quarter of xq/xk plus weights gates the first score tile.
    Q1, H1 = L // 4, L // 2

    def dma_x(dst, src, a, b):
        for c in range(DC):
            nc.sync.dma_start(out=dst[:, c, a:b], in_=src[:, c, a:b])

    dma_x(xqT, io["xq"], 0, Q1)
    nc.sync.dma_start(out=wq_sb, in_=io["wq"][:])
    nc.sync.dma_start(out=bq_sb, in_=io["bq"][:])
    dma_x(xkT, io["xk"], 0, Q1)
    nc.sync.dma_start(out=wk_sb, in_=io["wk"][:])
    nc.sync.dma_start(out=bk_sb, in_=io["bk"][:])
    nc.sync.dma_start(out=wv_sb, in_=io["wv"][:])
    dma_x(xvT, io["xv"], 0, Q1)      # v st0-3
    dma_x(xkT, io["xk"], Q1, H1)     # scores j4-7
    dma_x(xvT, io["xv"], Q1, H1)     # v st4-7
    dma_x(xvT, io["xv"], H1, S)      # v st8-15
    dma_x(xkT, io["xk"], H1, S)      # scores j8-15
    dma_x(xqT, io["xq"], Q1, H1)     # q(0,1)/q(1,1) for lq1
    dma_x(xqT, io["xq"], H1, L)      # lq2-3
    nc.sync.dma_start(out=wo_sb, in_=io["wo"][:])

    # ---- projections -----------------------------------------------------
    def proj_qk(xt, w_sb, b_sb, dst, ec, sq):
        # dst[:, ec, 512sq : 512sq+512] = (W.T @ X^T) + bias
        pp = ps_wk.tile([128, 512], F32, tag="work", name=f"pp{ec}_{sq}")
        for c in range(DC):
            mm(pp, lhsT=w_sb[:, c, ec * 128:(ec + 1) * 128],
               rhs=xt[:, c, sq * 512:(sq + 1) * 512],
               start=(c == 0), stop=(c == DC - 1))
        nc.vector.tensor_scalar_add(
            out=dst[:, ec, sq * 512:(sq + 1) * 512], in0=pp,
            scalar1=b_sb[:, ec:ec + 1])

    def proj_v(st):
        vp = ps_wk.tile([128, 512], F32, tag="work", name=f"vp{st}")[:, 0:EG]
        for c in range(DC):
            mm(vp, lhsT=xvT[:, c, st * 128:(st + 1) * 128],
               rhs=wv_sb[:, c, :], start=(c == 0), stop=(c == DC - 1))
        nc.vector.tensor_copy(
            out=v_sb[:, st, :, 0:64],
            in_=vp.rearrange("p (h e) -> p h e", h=HG))

    # Only what gates the first score tile runs up front; the rest of the
    # projection work is queued as fillers consumed one j-step at a time so
    # the in-order PE stream never stalls on late DMA chunks.
    proj_qk(xqT, wq_sb, bq_sb, qT, 0, 0)
    proj_qk(xkT, wk_sb, bk_sb, kT, 0, 0)
    proj_v(0)
    proj_v(1)

    def P(kind, *a):
        if kind == "q":
            return lambda: proj_qk(xqT, wq_sb, bq_sb, qT, *a)
        if kind == "k":
            return lambda: proj_qk(xkT, wk_sb, bk_sb, kT, *a)
        return lambda: proj_v(*a)

    # filler schedule per (lq, p) j-loop; None = no filler this j.  The
    # slot-j filler is EMITTED AFTER the j-th av matmul, so anything read
    # at step j must sit at slot <= j-1 (v st at slot st-2 for margin).
    # need-by: k(0,sq) by (lq0,p0) j=4sq; v st by (lq0,p0) j=st;
    # k(1,sq) by (lq*,p1) j=4sq; q(ec,sq) by (lq=sq, p=ec) j=0.
    fill = {
        (0, 0): [[P("k", 0, 1), P("v", 2)], P("v", 3), P("v", 4), P("v", 5),
                 [P("k", 0, 2), P("v", 6)], P("v", 7), P("v", 8),
                 [P("k", 0, 3), P("v", 9)], P("v", 10), P("v", 11),
                 P("v", 12), P("v", 13), [P("q", 1, 0), P("v", 14)],
                 P("v", 15), None, P("k", 1, 0)],
        (0, 1): [P("k", 1, 1), P("q", 0, 1), None, None,
                 P("k", 1, 2), P("q", 0, 2), None, None,
                 P("k", 1, 3), P("q", 0, 3), None, None,
                 P("q", 1, 1), None, None, None],
        (1, 0): [P("q", 1, 2), None, None, None,
                 P("q", 1, 3), None, None, None] + [None] * 8,
    }

    # ---- attention -------------------------------------------------------
    def out_proj(attnT, lt, i):
        op = ps_wk.tile([128, D], F32, tag="work", name=f"op_{lt}")
        for p in range(2):
            mm(op, lhsT=attnT[:, p, i * 128:(i + 1) * 128],
               rhs=wo_sb[:, p, :], start=(p == 0), stop=(p == 1))
        ob = ob_pool.tile([128, D], F16, tag="ob")
        nc.vector.tensor_copy(out=ob, in_=op)
        nc.sync.dma_start(out=io["out"][lt * 128:(lt + 1) * 128, :], in_=ob)

    pending = []   # deferred out-proj thunks
    for lq in range(LQ):
        l0 = lq * 512
        attnT = at_pool.tile([128, 2, 512], F16, tag="attnT")  # [64hh+e', pair, l]
        avs = at_pool.tile([64, HG, 512], F16, tag="avs")      # [e', h, l]
        # Z rows staged at partition 0: reciprocal_approx_fast (custom DVE
        # uop) reads the wrong row when its input AP has a partition offset.
        zrs = at_pool.tile([1, HG, 512], F32, tag="zrs")
        for p in range(2):                      # head pair
            av = [ps_av.tile([65, 512], F32, tag="av", name=f"av{lq}_{p}_{i}")
                  for i in range(2)]
            slots = fill.get((lq, p), [None] * ST)
            for j in range(ST):
                sc = ps_sc.tile([128, 2, 512], F32, tag="sc",
                                name=f"sc_{lq}_{p}_{j}")
                ep = e_pool.tile([128, 2, 512], F16, tag="ep")
                for hh in range(2):             # rows 0-63 / 64-127
                    o = hh * 64
                    mm(sc[:, hh, :],
                       lhsT=kT[o:o + 64, p, j * 128:(j + 1) * 128],
                       rhs=qT[o:o + 64, p, l0:l0 + 512],
                       start=True, stop=True, tile_position=(o, 0))
                nc.scalar.activation(out=ep, in_=sc,
                                     func=mybir.ActivationFunctionType.Exp,
                                     bias=shift_col[:, 0:1], scale=1.0)
                for hh in range(2):
                    mm(av[hh], lhsT=v_sb[:, j, 2 * p + hh, :],
                       rhs=ep[:, hh, :], start=(j == 0), stop=(j == ST - 1))
                f = slots[j]
                if f is not None:
                    for g in (f if isinstance(f, list) else [f]):
                        g()
                elif pending:
                    pending.pop(0)()
            # drain av PSUM fast: Z row -> zrs (partition 0), numerator -> f16
            for hh in range(2):
                h = 2 * p + hh
                nc.vector.tensor_copy(out=zrs[:, h, :], in_=av[hh][64:65, :])
                nc.vector.tensor_copy(out=avs[:, h, :], in_=av[hh][0:64, :])
            # 1/Z off the PSUM critical path (SBUF in/out)
            for hh in range(2):
                h = 2 * p + hh
                rrow = z_pool.tile([1, 512], F32, tag="rrow")
                nc.vector.reciprocal_approx_fast(out=rrow, in_=zrs[:, h, :])
                zbb = z_pool.tile([64, 512], F32, tag="zbb")
                nc.gpsimd.partition_broadcast(zbb, rrow)
                nc.vector.tensor_mul(
                    out=attnT[64 * hh:64 * hh + 64, p, :],
                    in0=avs[:, h, :], in1=zbb)
        # output projection (head pairs stacked, K=128): deferred into the
        # next l-quarter's j-loops so it never blocks the in-order PE
        # stream at the quarter boundary; the last quarter runs inline.
        for i in range(4):
            pending.append(lambda a=attnT, lt=lq * 4 + i, i=i: out_proj(a, lt, i))
    while pending:
        pending.pop(0)()


def build_nc():
    nc = bacc.Bacc()
    io = {}
    io["xq"] = nc.declare_dram_parameter("xq", [128, DC, L], F16, isOutput=False)
    io["xk"] = nc.declare_dram_parameter("xk", [128, DC, S], F16, isOutput=False)
    io["xv"] = nc.declare_dram_parameter("xv", [128, DC, S], F16, isOutput=False)
    io["wq"] = nc.declare_dram_parameter("wq", [128, DC, EG], F16, isOutput=False)
    io["wk"] = nc.declare_dram_parameter("wk", [128, DC, EG], F16, isOutput=False)
    io["wv"] = nc.declare_dram_parameter("wv", [128, DC, EG], F16, isOutput=False)
    io["wo"] = nc.declare_dram_parameter("wo", [128, 2, D], F16, isOutput=False)
    io["bq"] = nc.declare_dram_parameter("bq", [128, 2], F32, isOutput=False)
    io["bk"] = nc.declare_dram_parameter("bk", [128, 2], F32, isOutput=False)
    io["out"] = nc.declare_dram_parameter("out", [L, D], F16, isOutput=True)
    with tile.TileContext(nc) as tc:
        with ExitStack() as ctx:
            _emit(ctx, tc, io)
    nc.compile()
    return nc


_NC = None


def _get_nc():
    global _NC
    if _NC is None:
        _NC = build_nc()
    return _NC


def _chunk_w(w):
    """[512, n] -> [128, 4, n] fp16:  [p, c, :] = w[128c+p, :]"""
    n = w.shape[1]
    return np.ascontiguousarray(
        w.reshape(DC, 128, n).transpose(1, 0, 2), dtype=np.float16)


def _xt(x):
    """[2048, 512] f32 -> [128, 4, 2048] fp16:  [p, c, l] = x[l, 128c+p]"""
    return np.ascontiguousarray(
        x.T.reshape(DC, 128, -1).transpose(1, 0, 2), dtype=np.float16)


def make_in_maps(queries, keys, values, tau, Wq, bq, Wk, bk, Wv, bv, Wo):
    in_maps = []
    for c in range(N_CORES):
        b, g = c // 2, c % 2
        e0 = g * EG
        f = np.float32(SCALE * tau[b])
        wq = _chunk_w(Wq[:, e0:e0 + EG] * f)
        wk = _chunk_w(Wk[:, e0:e0 + EG])
        wv = _chunk_w(Wv[:, e0:e0 + EG])
        wo = np.ascontiguousarray(
            Wo[e0:e0 + EG, :].reshape(2, 128, D).transpose(1, 0, 2),
            dtype=np.float16)
        in_maps.append({
            "xq": _xt(queries[b]),
            "xk": _xt(keys[b]),
            "xv": _xt(values[b]),
            "wq": wq, "wk": wk, "wv": wv, "wo": wo,
            "bq": np.ascontiguousarray(
                (bq[e0:e0 + EG] * f).reshape(2, 128).T, dtype=np.float32),
            "bk": np.ascontiguousarray(
                bk[e0:e0 + EG].reshape(2, 128).T, dtype=np.float32),
        })
    return in_maps


def kernel(queries, keys, values, tau, delta, Wq, bq, Wk, bk, Wv, bv, Wo, bo,
           **_unused):
    queries = np.asarray(queries, dtype=np.float32)
    keys = np.asarray(keys, dtype=np.float32)
    values = np.asarray(values, dtype=np.float32)
    tau = np.asarray(tau, dtype=np.float32)
    Wq, bq = np.asarray(Wq, np.float32), np.asarray(bq, np.float32)
    Wk, bk = np.asarray(Wk, np.float32), np.asarray(bk, np.float32)
    Wv, bv = np.asarray(Wv, np.float32), np.asarray(bv, np.float32)
    Wo, bo = np.asarray(Wo, np.float32), np.asarray(bo, np.float32)

    nc = _get_nc()
    in_maps = make_in_maps(queries, keys, values, tau, Wq, bq, Wk, bk, Wv, bv, Wo)
    res = run_bass_kernel_spmd(nc, in_maps, list(range(N_CORES)))
    # attn rows sum to 1 -> +bv flows through Wo as a constant row; + bo.
    const_row = (bv @ Wo + bo).astype(np.float32)  # [512]
    out = np.empty((B, L, D), dtype=np.float32)
    for b in range(B):
        out[b] = res.results[2 * b]["out"].astype(np.float32) \
            + res.results[2 * b + 1]["out"].astype(np.float32) + const_row
    return out


if __name__ == "__main__":
    nc = build_nc()
    print("built OK")
